# revision 18
# baseline (speedup 1.0000x reference)
"""Trainium2 Bass kernel for the ALIF leaky-RNN problem (nn_ALIF).

Reference computation (T sequential steps):
    w_dale_t = (mask * relu(weight_rec) * weight_sign[None, :]).T   # [pre, post]
    r_t   = sigmoid(3 u_t - 2)                                      # [B, N]
    drive = r_t @ w_dale_t + MUE + ext_t
    u_{t+1} = u_t (1 - dt) + drive * dt
    out_t = r_t @ weight_out
    outputs: usave [T, B, N] (= u_1..u_T), outsave [T, B, NOUT]

Distribution: column-parallel over the N=2048 neurons across 8 cores
(256 post-synaptic neurons per core), full batch B=32 everywhere.
Neuron-major layout: a per-core tile is [128 part, 64] with column
32*jj + b = (chunk jj, batch b), neuron = core*256 + 128*jj + p.

Per step, each core:
  - PE: 32 fp16 matmuls accumulate r_t @ W_shard into PSUM [128, 64]
    (W block stationary [128,128], gathered-r moving [128,32]); plus a
    2-matmul partial readout into a [2, 32] PSUM.
  - DVE: h = u*(1-dt) + dt*(MUE+ext_t) (prepared while PE works), then
    u_{t+1} = dt*psum + h.
  - ACT: r16_{t+1} = sigmoid(3 u_{t+1} - 2) in fp16.
  - gpsimd: broadcasts the r16 tile to all 8 cores with 8 single-slot
    remote_dma_broadcast calls (XOR-relative dests). Receiver r's G-block
    k holds core (r XOR k)'s chunk; each core's W copy is pre-permuted
    host-side to match, so no runtime indexing is needed.

The double-buffered gather tile G (even/odd parity, one arrival semaphore
per parity) is race-free because each step is a full barrier: no core can
broadcast step t+2 before every core has received all of step t.  A
one-time collective AllReduce barrier at startup guarantees every core's
NEFF is loaded before any remote DMA lands (launch skew protection).
"""

from contextlib import ExitStack

import numpy as np

import concourse.bass as bass
import concourse.bacc as bacc
import concourse.mybir as mybir

NCORES = 8
N = 2048
B = 32
T = 500
NOUT = 2
S = N // NCORES          # 256 neurons per core
MUE = 0.5

F32 = mybir.dt.float32
F16 = mybir.dt.float16
AF = mybir.ActivationFunctionType
ALU = mybir.AluOpType


def build_nc(t_steps=T, dt_scalar=0.1, debug=False):
    """Build the SPMD bass program (identical on all 8 cores).

    dt_scalar: if not None, dt_tausinv is uniform with this value and the
    DVE update uses immediate scalars on full [128, 64] tiles; if None,
    per-partition scalar APs are used per 32-column chunk (general dt).
    """
    assert t_steps % 2 == 0
    nbody = t_steps // 2
    uniform = dt_scalar is not None

    nc = bacc.Bacc(target_bir_lowering=False, debug=debug)

    # ---- DRAM parameters (per-core content differs via in_maps) ----
    wrec = nc.dram_tensor("wrec", [128, 32 * 128], F16, kind="ExternalInput")
    wout = nc.dram_tensor("wout", [128, 4], F16, kind="ExternalInput")
    scl = nc.dram_tensor("scl", [128, 6], F32, kind="ExternalInput")
    u0t = nc.dram_tensor("u0t", [128, 64], F32, kind="ExternalInput")
    exts = nc.dram_tensor("exts", [t_steps * 128, 64], F32, kind="ExternalInput")
    usave = nc.dram_tensor("usave", [t_steps * 128, 64], F32, kind="ExternalOutput")
    osave = nc.dram_tensor("osave", [t_steps * 2, 32], F32, kind="ExternalOutput")
    # startup-barrier bounce buffers (content irrelevant)
    bar_in = nc.dram_tensor("bar_in", [1, 8], F32)
    bar_out = nc.dram_tensor("bar_out", [1, 8], F32, addr_space="Shared")

    with ExitStack() as _es:
        ec = _es.enter_context
        # ---- SBUF ----
        w_sb = ec(nc.sbuf_tensor("w_sb", [128, 32 * 128], F16))
        wo_sb = ec(nc.sbuf_tensor("wo_sb", [128, 4], F16))
        scl_sb = ec(nc.sbuf_tensor("scl_sb", [128, 6], F32))
        u_a = ec(nc.sbuf_tensor("u_a", [128, 64], F32))
        u_b = ec(nc.sbuf_tensor("u_b", [128, 64], F32))
        h_sb = ec(nc.sbuf_tensor("h_sb", [128, 64], F32))
        r16_a = ec(nc.sbuf_tensor("r16_a", [128, 64], F16))
        r16_b = ec(nc.sbuf_tensor("r16_b", [128, 64], F16))
        g_a = ec(nc.sbuf_tensor("g_a", [128, 512], F16))
        g_b = ec(nc.sbuf_tensor("g_b", [128, 512], F16))
        ext_a = ec(nc.sbuf_tensor("ext_a", [128, 64], F32))
        ext_b = ec(nc.sbuf_tensor("ext_b", [128, 64], F32))
        osv_a = ec(nc.sbuf_tensor("osv_a", [2, 32], F32))
        osv_b = ec(nc.sbuf_tensor("osv_b", [2, 32], F32))
        zr_sb = ec(nc.sbuf_tensor("zr_sb", [1, 8], F32))
        # ---- PSUM ----
        ps = ec(nc.psum_tensor("ps", [128, 64], F32))
        po_a = ec(nc.psum_tensor("po_a", [2, 32], F32))
        po_b = ec(nc.psum_tensor("po_b", [2, 32], F32))
        # ---- semaphores ----
        init_sem = ec(nc.semaphore("init_sem"))
        cc_sem = ec(nc.semaphore("cc_sem"))
        zz_sem = ec(nc.semaphore("zz_sem"))
        sem_ga = ec(nc.semaphore("sem_ga"))
        sem_gb = ec(nc.semaphore("sem_gb"))
        prep_sem = ec(nc.semaphore("prep_sem"))
        lsem_a = ec(nc.semaphore("lsem_a"))
        lsem_b = ec(nc.semaphore("lsem_b"))
        act1_sem = ec(nc.semaphore("act1_sem"))
        psum_sem = ec(nc.semaphore("psum_sem"))
        dveu_sem = ec(nc.semaphore("dveu_sem"))
        rdout_sem = ec(nc.semaphore("rdout_sem"))
        dvo_sem = ec(nc.semaphore("dvo_sem"))
        exta_sem = ec(nc.semaphore("exta_sem"))
        extb_sem = ec(nc.semaphore("extb_sem"))
        usave_sem = ec(nc.semaphore("usave_sem"))
        osv_dma_sem = ec(nc.semaphore("osv_dma_sem"))
        block = ec(nc.Block())

        u_t = [u_a, u_b]
        r16_t = [r16_a, r16_b]
        g_t = [g_a, g_b]
        ext_t = [ext_a, ext_b]
        osv_t = [osv_a, osv_b]
        po_t = [po_a, po_b]
        sem_g = [sem_ga, sem_gb]
        lsem = [lsem_a, lsem_b]
        ext_sem = [exta_sem, extb_sem]

        # W block (k, j, jj) lives at columns [((k*2+j)*2+jj)*128, +128).
        def wblk(k, j, jj):
            base = ((k * 2 + j) * 2 + jj) * 128
            return w_sb[:, base:base + 128]

        # per-chunk column slice of a [128, 64] tile
        def ch(tile, jj):
            return tile[:, 32 * jj:32 * jj + 32]

        # ---------------- sync engine: all HBM DMA traffic ----------------
        @block.sync
        def _(sync):
            sync.dma_start(w_sb[:, :], wrec[:, :]).then_inc(init_sem, 16)
            sync.dma_start(wo_sb[:, :], wout[:, :]).then_inc(init_sem, 16)
            sync.dma_start(scl_sb[:, :], scl[:, :]).then_inc(init_sem, 16)
            sync.dma_start(u_a[:, :], u0t[:, :]).then_inc(init_sem, 16)
            sync.dma_start(ext_a[:, :], exts[0:128, :]).then_inc(exta_sem, 16)
            sync.dma_start(ext_b[:, :], exts[128:256, :]).then_inc(extb_sem, 16)

            for t in range(t_steps):
                par = t % 2
                # usave[t] = u_{t+1}, produced by DVE into u_t[(t+1)%2]
                sync.wait_ge(dveu_sem, t + 1)
                sync.dma_start(
                    usave[t * 128:(t + 1) * 128, :],
                    u_t[(t + 1) % 2][:, :],
                ).then_inc(usave_sem, 16)
                # prefetch ext for step t+2 into the tile step t consumed
                if t + 2 < t_steps:
                    sync.dma_start(
                        ext_t[par][:, :],
                        exts[(t + 2) * 128:(t + 3) * 128, :],
                    ).then_inc(ext_sem[par], 16)
                # outsave[t]
                sync.wait_ge(dvo_sem, t + 1)
                sync.dma_start(
                    osave[t * 2:(t + 1) * 2, :],
                    osv_t[par][:, :],
                ).then_inc(osv_dma_sem, 16)

            sync.wait_ge(usave_sem, 16 * t_steps)
            sync.wait_ge(osv_dma_sem, 16 * t_steps)

        # ---------------- tensor engine: all matmuls ----------------
        @block.tensor
        def _(tensor):
            def emit_step(par, th_act1, th_g, th_dvo, th_psfree):
                # partial readout for step t from local r16
                tensor.wait_ge(act1_sem, th_act1)
                tensor.wait_ge(dvo_sem, th_dvo)
                tensor.matmul(po_t[par][:, :], wo_sb[:, 0:2],
                              r16_t[par][:, 0:32], start=True, stop=False)
                tensor.matmul(po_t[par][:, :], wo_sb[:, 2:4],
                              r16_t[par][:, 32:64], start=False, stop=True,
                              ).then_inc(rdout_sem, 1)
                # recurrent accumulation: psum = r_t @ W
                tensor.wait_ge(dveu_sem, th_psfree)
                tensor.wait_ge(sem_g[par], th_g)
                first = True
                for jj in range(2):
                    for k in range(NCORES):
                        for j in range(2):
                            last = (jj == 1 and k == NCORES - 1 and j == 1)
                            mm = tensor.matmul(
                                ch(ps, jj),
                                wblk(k, j, jj),
                                g_t[par][:, 64 * k + 32 * j:64 * k + 32 * j + 32],
                                start=first, stop=last)
                            first = False
                            if last:
                                mm.then_inc(psum_sem, 1)

            tensor.wait_ge(init_sem, 64)
            with (
                tensor.register("r_act1") as r_act1,
                tensor.register("r_ga") as r_ga,
                tensor.register("r_gb") as r_gb,
                tensor.register("r_dvo") as r_dvo,
                tensor.register("r_psf") as r_psf,
            ):
                tensor.reg_mov(r_act1, 1)
                tensor.reg_mov(r_ga, 16)
                tensor.reg_mov(r_gb, 16)
                tensor.reg_mov(r_dvo, 0)
                tensor.reg_mov(r_psf, 0)
                with tensor.Fori(0, nbody):
                    emit_step(0, r_act1, r_ga, r_dvo, r_psf)
                    tensor.reg_add(r_act1, r_act1, 1)
                    tensor.reg_add(r_psf, r_psf, 1)
                    emit_step(1, r_act1, r_gb, r_dvo, r_psf)
                    tensor.reg_add(r_act1, r_act1, 1)
                    tensor.reg_add(r_psf, r_psf, 1)
                    tensor.reg_add(r_ga, r_ga, 16)
                    tensor.reg_add(r_gb, r_gb, 16)
                    tensor.reg_add(r_dvo, r_dvo, 2)

        # -------- vector engine (DVE): leaky integration + readout copy ----
        @block.vector
        def _(vector):
            def dve_step(par, regs):
                nxt = 1 - par
                # h = u_t*(1-dt) + dt*(mue+ext_t)   (off critical path)
                vector.wait_ge(ext_sem[par], regs["ext"][par])
                if uniform:
                    vector.scalar_tensor_tensor(
                        h_sb[:, :], u_t[par][:, :], 1.0 - dt_scalar,
                        ext_t[par][:, :], ALU.mult, ALU.add)
                else:
                    for jj in range(2):
                        vector.scalar_tensor_tensor(
                            ch(h_sb, jj), ch(u_t[par], jj),
                            scl_sb[:, 2 + jj:3 + jj], ch(ext_t[par], jj),
                            ALU.mult, ALU.add)
                vector.drain()
                # u_{t+1} = dt*psum + h   (critical path)
                vector.wait_ge(psum_sem, regs["psum"])
                vector.wait_ge(usave_sem, regs["usave"])
                if uniform:
                    vector.scalar_tensor_tensor(
                        u_t[nxt][:, :], ps[:, :], dt_scalar, h_sb[:, :],
                        ALU.mult, ALU.add).then_inc(dveu_sem, 1)
                else:
                    vector.scalar_tensor_tensor(
                        ch(u_t[nxt], 0), ch(ps, 0), scl_sb[:, 0:1],
                        ch(h_sb, 0), ALU.mult, ALU.add)
                    vector.scalar_tensor_tensor(
                        ch(u_t[nxt], 1), ch(ps, 1), scl_sb[:, 1:2],
                        ch(h_sb, 1), ALU.mult, ALU.add).then_inc(dveu_sem, 1)
                vector.drain()
                # readout copy (off critical path)
                vector.wait_ge(rdout_sem, regs["rd"])
                vector.wait_ge(osv_dma_sem, regs["osv"])
                vector.tensor_copy(osv_t[par][:, :], po_t[par][:, :]
                                   ).then_inc(dvo_sem, 1)

            vector.wait_ge(init_sem, 64)
            with (
                vector.register("r_exa") as r_exa,
                vector.register("r_exb") as r_exb,
                vector.register("r_ps") as r_ps,
                vector.register("r_us") as r_us,
                vector.register("r_rd") as r_rd,
                vector.register("r_ov") as r_ov,
            ):
                vector.reg_mov(r_exa, 16)
                vector.reg_mov(r_exb, 16)
                vector.reg_mov(r_ps, 1)
                vector.reg_mov(r_us, 0)
                vector.reg_mov(r_rd, 1)
                vector.reg_mov(r_ov, 0)
                regs = {"ext": [r_exa, r_exb], "psum": r_ps, "usave": r_us,
                        "rd": r_rd, "osv": r_ov}
                with vector.Fori(0, nbody):
                    dve_step(0, regs)
                    vector.reg_add(r_ps, r_ps, 1)
                    vector.reg_add(r_us, r_us, 16)
                    vector.reg_add(r_rd, r_rd, 1)
                    vector.reg_add(r_ov, r_ov, 16)
                    dve_step(1, regs)
                    vector.reg_add(r_ps, r_ps, 1)
                    vector.reg_add(r_us, r_us, 16)
                    vector.reg_add(r_rd, r_rd, 1)
                    vector.reg_add(r_ov, r_ov, 16)
                    vector.reg_add(r_exa, r_exa, 16)
                    vector.reg_add(r_exb, r_exb, 16)

        # ---------------- scalar engine (ACT): sigmoid ----------------
        @block.scalar
        def _(scalar):
            scalar.wait_ge(init_sem, 64)
            # r_0 = sigmoid(3 u0 - 2), fp16
            scalar.activation(r16_a[:, :], u_a[:, :], AF.Sigmoid,
                              bias=scl_sb[:, 4:5], scale=3.0,
                              ).then_inc(act1_sem, 1)
            with (
                scalar.register("r_dvu") as r_dvu,
                scalar.register("r_lsa") as r_lsa,
                scalar.register("r_lsb") as r_lsb,
            ):
                scalar.reg_mov(r_dvu, 1)
                scalar.reg_mov(r_lsa, 128)
                scalar.reg_mov(r_lsb, 0)
                with scalar.Fori(0, nbody):
                    for par in range(2):
                        nxt = 1 - par
                        scalar.wait_ge(dveu_sem, r_dvu)
                        # r16[nxt] was last read by the broadcast of step t-1
                        # (same parity); wait for those sends (WAR edge)
                        scalar.wait_ge(lsem[nxt], [r_lsa, r_lsb][nxt])
                        scalar.activation(r16_t[nxt][:, :], u_t[nxt][:, :],
                                          AF.Sigmoid, bias=scl_sb[:, 4:5],
                                          scale=3.0).then_inc(act1_sem, 1)
                        scalar.reg_add(r_dvu, r_dvu, 1)
                    scalar.reg_add(r_lsa, r_lsa, 128)
                    scalar.reg_add(r_lsb, r_lsb, 128)

        # ---------------- gpsimd: barrier + remote broadcast of r16 --------
        @block.gpsimd
        def _(gpsimd):
            def emit_bcast_preps(src_tile, dst_g, rsem, ls):
                # The Q7 XORs dests with its own PHYSICAL tpb id; the logical
                # -> physical NC map on trn2 is [0,1,2,3,6,7,4,5], so a
                # logical-XOR-k hop needs physical delta k^2 when bit 2 flips
                # (uniform across senders since p(s)^p(s^k) == k^2 for k>=4).
                for k in range(NCORES):
                    rdests = [None] * NCORES
                    rdests[k] = (0, k ^ 2 if k >= 4 else k)
                    gpsimd.remote_dma_broadcast(
                        dst_g[:, 64 * k:64 * k + 64],
                        src_tile[:, :],
                        rsem,
                        ls,
                        rdests=rdests,
                    ).then_inc(prep_sem, 1)

            # startup barrier: guarantees all 8 cores' NEFFs are loaded and
            # running before any remote_dma traffic is emitted
            gpsimd.memset(zr_sb[:, :], 0.0)
            gpsimd.drain()
            gpsimd.dma_start(bar_in[:, :], zr_sb[:, :]).then_inc(zz_sem, 16)
            gpsimd.wait_ge(zz_sem, 16)
            gpsimd.collective_compute(
                "AllReduce",
                ALU.add,
                replica_groups=[list(range(NCORES))],
                ins=[bar_in.ap().opt()],
                outs=[bar_out.ap().opt()],
            ).then_inc(cc_sem)
            gpsimd.wait_ge(cc_sem, 1)

            # step-0 broadcast: r16_a -> g_a on every core
            emit_bcast_preps(r16_a, g_a, sem_ga, lsem_a)
            gpsimd.wait_ge(prep_sem, 8)
            gpsimd.wait_ge(act1_sem, 1)
            gpsimd.trigger_dma(count=8)

            with (
                gpsimd.register("r_prep") as r_prep,
                gpsimd.register("r_act1") as r_act1g,
            ):
                gpsimd.reg_mov(r_prep, 16)
                gpsimd.reg_mov(r_act1g, 2)
                with gpsimd.Fori(0, nbody):
                    for par in range(2):
                        nxt = 1 - par
                        # broadcast r16 of step t+1 into G[(t+1)%2]
                        emit_bcast_preps(r16_t[nxt], g_t[nxt], sem_g[nxt],
                                         lsem[nxt])
                        gpsimd.wait_ge(prep_sem, r_prep)
                        gpsimd.wait_ge(act1_sem, r_act1g)
                        gpsimd.trigger_dma(count=8)
                        gpsimd.reg_add(r_prep, r_prep, 8)
                        gpsimd.reg_add(r_act1g, r_act1g, 1)
            # drain the final (unused) broadcast before finishing
            gpsimd.wait_ge(sem_ga, 16 * (nbody + 1))

    if not nc.is_finalized():
        nc.finalize()
    return nc


# ======================= host-side packing =======================

def _to_tile64(x):
    """[..., B=32, 256] per-core slice -> [..., 128, 64] neuron-major tile."""
    return np.ascontiguousarray(
        x.reshape(x.shape[:-2] + (32, 2, 128)).swapaxes(-1, -3).reshape(
            x.shape[:-2] + (128, 64)))


def _from_tile64(x):
    """[..., 128, 64] tile -> [..., 32, 256]."""
    return np.ascontiguousarray(
        x.reshape(x.shape[:-2] + (128, 2, 32)).swapaxes(-1, -3).reshape(
            x.shape[:-2] + (32, 256)))


def make_in_maps(u0, ext, weight_rec, mask, weight_sign, weight_out,
                 dt_tausinv, t_steps=T):
    u0 = np.asarray(u0, np.float32)
    ext = np.asarray(ext, np.float32)[:t_steps]
    w_dale_t = (np.asarray(mask, np.float32)
                * np.maximum(np.asarray(weight_rec, np.float32), 0.0)
                * np.asarray(weight_sign, np.float32)[None, :]).T  # [pre, post]
    wout_f = np.asarray(weight_out, np.float32)
    dt = np.asarray(dt_tausinv, np.float32).reshape(-1)            # [N]

    in_maps = []
    for c in range(NCORES):
        # recurrent weights, permuted to match the XOR-relative gather layout
        wrec_c = np.empty((128, 32 * 128), np.float32)
        for k in range(NCORES):
            src = (c ^ k) * S
            for j in range(2):
                for jj in range(2):
                    base = ((k * 2 + j) * 2 + jj) * 128
                    wrec_c[:, base:base + 128] = w_dale_t[
                        src + 128 * j: src + 128 * (j + 1),
                        c * S + 128 * jj: c * S + 128 * (jj + 1)]
        # readout weights: cols [2j, 2j+2) = chunk j
        wout_c = np.empty((128, 4), np.float32)
        for j in range(2):
            wout_c[:, 2 * j:2 * j + 2] = wout_f[c * S + 128 * j: c * S + 128 * (j + 1)]
        # per-partition dt scalars (used only when dt is non-uniform) + consts
        scl_c = np.empty((128, 6), np.float32)
        for jj in range(2):
            dloc = dt[c * S + 128 * jj: c * S + 128 * (jj + 1)]
            scl_c[:, jj] = dloc
            scl_c[:, 2 + jj] = 1.0 - dloc
        scl_c[:, 4] = -2.0
        scl_c[:, 5] = 0.0

        dt_t = _to_tile64(dt[None, c * S:(c + 1) * S].repeat(32, axis=0))
        u0t_c = _to_tile64(u0[:, c * S:(c + 1) * S])
        # ext tile pre-scaled: dt * (MUE + ext_t)
        exts_c = dt_t[None] * _to_tile64(MUE + ext[:, :, c * S:(c + 1) * S])

        in_maps.append(dict(
            wrec=wrec_c.astype(np.float16),
            wout=wout_c.astype(np.float16),
            scl=scl_c,
            u0t=u0t_c,
            exts=np.ascontiguousarray(exts_c, np.float32).reshape(
                t_steps * 128, 64),
        ))
    return in_maps


def unpack_outputs(results, t_steps=T):
    usave = np.empty((t_steps, B, N), np.float32)
    osum = np.zeros((t_steps, 2, 32), np.float32)
    for c in range(NCORES):
        uc = np.asarray(results[c]["usave"], np.float32).reshape(t_steps, 128, 64)
        usave[:, :, c * S:(c + 1) * S] = _from_tile64(uc)
        osum += np.asarray(results[c]["osave"], np.float32).reshape(t_steps, 2, 32)
    outsave = np.ascontiguousarray(osum.transpose(0, 2, 1))
    return usave, outsave


_CACHED = {}


def kernel(u0, ext, weight_rec, mask, weight_sign, weight_out, dt_tausinv):
    from concourse.bass_utils import run_bass_kernel_spmd

    dt = np.asarray(dt_tausinv, np.float32).reshape(-1)
    dt_scalar = float(dt[0]) if np.all(dt == dt[0]) else None
    in_maps = make_in_maps(u0, ext, weight_rec, mask, weight_sign,
                           weight_out, dt_tausinv)
    key = (T, dt_scalar)
    if key not in _CACHED:
        _CACHED[key] = build_nc(t_steps=T, dt_scalar=dt_scalar)
    res = run_bass_kernel_spmd(_CACHED[key], in_maps,
                               core_ids=list(range(NCORES)))
    return unpack_outputs(res.results)


# revision 20
# speedup vs baseline: 4.0489x; 4.0489x over previous
"""Trainium2 Bass kernel for the ALIF leaky-RNN problem (nn_ALIF).

Reference computation (T sequential steps):
    w_dale_t = (mask * relu(weight_rec) * weight_sign[None, :]).T   # [pre, post]
    r_t   = sigmoid(3 u_t - 2)                                      # [B, N]
    drive = r_t @ w_dale_t + MUE + ext_t
    u_{t+1} = u_t (1 - dt) + drive * dt
    out_t = r_t @ weight_out
    outputs: usave [T, B, N] (= u_1..u_T), outsave [T, B, NOUT]

Distribution: column-parallel over the N=2048 neurons across 8 cores
(256 post-synaptic neurons per core), full batch B=32 everywhere.
Neuron-major layout: a per-core tile is [128 part, 64] with column
32*jj + b = (chunk jj, batch b), neuron = core*256 + 128*jj + p.

Per step, each core:
  - PE: 32 fp16 matmuls accumulate r_t @ W_shard into PSUM [128, 64]
    (W block stationary [128,128], gathered-r moving [128,32]); plus a
    2-matmul partial readout into a [2, 32] PSUM.
  - DVE: h = u*(1-dt) + dt*(MUE+ext_t) (prepared while PE works), then
    u_{t+1} = dt*psum + h.
  - ACT: r16_{t+1} = sigmoid(3 u_{t+1} - 2) in fp16.
  - comm: r16 goes SBUF -> DRAM bounce -> collective AllGather (ncfw)
    -> DRAM gathered -> SBUF G tile [128, 8*64], double-buffered by step
    parity.  The AllGather is also the per-step barrier across cores.
"""

from contextlib import ExitStack

import numpy as np

import concourse.bass as bass
import concourse.bacc as bacc
import concourse.mybir as mybir

NCORES = 8
N = 2048
B = 32
T = 500
NOUT = 2
S = N // NCORES          # 256 neurons per core
MUE = 0.5

F32 = mybir.dt.float32
F16 = mybir.dt.float16
AF = mybir.ActivationFunctionType
ALU = mybir.AluOpType


def build_nc(t_steps=T, dt_scalar=0.1, debug=False):
    """Build the SPMD bass program (identical on all 8 cores).

    dt_scalar: if not None, dt_tausinv is uniform with this value and the
    DVE update uses immediate scalars on full [128, 64] tiles; if None,
    per-partition scalar APs are used per 32-column chunk (general dt).
    """
    assert t_steps % 2 == 0
    nbody = t_steps // 2
    uniform = dt_scalar is not None

    nc = bacc.Bacc(target_bir_lowering=False, debug=debug)

    # ---- DRAM parameters (per-core content differs via in_maps) ----
    wrec = nc.dram_tensor("wrec", [128, 32 * 128], F16, kind="ExternalInput")
    wout = nc.dram_tensor("wout", [128, 4], F16, kind="ExternalInput")
    scl = nc.dram_tensor("scl", [128, 6], F32, kind="ExternalInput")
    u0t = nc.dram_tensor("u0t", [128, 64], F32, kind="ExternalInput")
    exts = nc.dram_tensor("exts", [t_steps * 128, 64], F32, kind="ExternalInput")
    usave = nc.dram_tensor("usave", [t_steps * 128, 64], F32, kind="ExternalOutput")
    osave = nc.dram_tensor("osave", [t_steps * 2, 32], F32, kind="ExternalOutput")
    # collective bounce buffers (internal DRAM; AG output must be Shared)
    bin_a = nc.dram_tensor("bin_a", [128, 64], F16)
    bin_b = nc.dram_tensor("bin_b", [128, 64], F16)
    bout_a = nc.dram_tensor("bout_a", [1024, 64], F16, addr_space="Shared")
    bout_b = nc.dram_tensor("bout_b", [1024, 64], F16, addr_space="Shared")

    with ExitStack() as _es:
        ec = _es.enter_context
        # ---- SBUF ----
        w_sb = ec(nc.sbuf_tensor("w_sb", [128, 32 * 128], F16))
        wo_sb = ec(nc.sbuf_tensor("wo_sb", [128, 4], F16))
        scl_sb = ec(nc.sbuf_tensor("scl_sb", [128, 6], F32))
        u_a = ec(nc.sbuf_tensor("u_a", [128, 64], F32))
        u_b = ec(nc.sbuf_tensor("u_b", [128, 64], F32))
        h_sb = ec(nc.sbuf_tensor("h_sb", [128, 64], F32))
        r16_a = ec(nc.sbuf_tensor("r16_a", [128, 64], F16))
        r16_b = ec(nc.sbuf_tensor("r16_b", [128, 64], F16))
        g_a = ec(nc.sbuf_tensor("g_a", [128, 512], F16))
        g_b = ec(nc.sbuf_tensor("g_b", [128, 512], F16))
        ext_a = ec(nc.sbuf_tensor("ext_a", [128, 64], F32))
        ext_b = ec(nc.sbuf_tensor("ext_b", [128, 64], F32))
        osv_a = ec(nc.sbuf_tensor("osv_a", [2, 32], F32))
        osv_b = ec(nc.sbuf_tensor("osv_b", [2, 32], F32))
        # ---- PSUM ----
        ps = ec(nc.psum_tensor("ps", [128, 64], F32))
        po_a = ec(nc.psum_tensor("po_a", [2, 32], F32))
        po_b = ec(nc.psum_tensor("po_b", [2, 32], F32))
        # ---- semaphores ----
        init_sem = ec(nc.semaphore("init_sem"))
        ccsem = ec(nc.semaphore("ccsem"))
        sem_ga = ec(nc.semaphore("sem_ga"))
        sem_gb = ec(nc.semaphore("sem_gb"))
        b1sem_a = ec(nc.semaphore("b1sem_a"))
        b1sem_b = ec(nc.semaphore("b1sem_b"))
        act1_sem = ec(nc.semaphore("act1_sem"))
        psum_sem = ec(nc.semaphore("psum_sem"))
        dveu_sem = ec(nc.semaphore("dveu_sem"))
        rdout_sem = ec(nc.semaphore("rdout_sem"))
        dvo_sem = ec(nc.semaphore("dvo_sem"))
        exta_sem = ec(nc.semaphore("exta_sem"))
        extb_sem = ec(nc.semaphore("extb_sem"))
        usave_sem = ec(nc.semaphore("usave_sem"))
        osv_dma_sem = ec(nc.semaphore("osv_dma_sem"))
        block = ec(nc.Block())

        u_t = [u_a, u_b]
        r16_t = [r16_a, r16_b]
        g_t = [g_a, g_b]
        ext_t = [ext_a, ext_b]
        osv_t = [osv_a, osv_b]
        po_t = [po_a, po_b]
        sem_g = [sem_ga, sem_gb]
        b1sem = [b1sem_a, b1sem_b]
        bin_t = [bin_a, bin_b]
        bout_t = [bout_a, bout_b]
        ext_sem = [exta_sem, extb_sem]

        # W block (k, j, jj) lives at columns [((k*2+j)*2+jj)*128, +128).
        def wblk(k, j, jj):
            base = ((k * 2 + j) * 2 + jj) * 128
            return w_sb[:, base:base + 128]

        # per-chunk column slice of a [128, 64] tile
        def ch(tile, jj):
            return tile[:, 32 * jj:32 * jj + 32]

        # ---------------- sync engine: all HBM DMA traffic ----------------
        @block.sync
        def _(sync):
            sync.dma_start(w_sb[:, :], wrec[:, :]).then_inc(init_sem, 16)
            sync.dma_start(wo_sb[:, :], wout[:, :]).then_inc(init_sem, 16)
            sync.dma_start(scl_sb[:, :], scl[:, :]).then_inc(init_sem, 16)
            sync.dma_start(u_a[:, :], u0t[:, :]).then_inc(init_sem, 16)
            sync.dma_start(ext_a[:, :], exts[0:128, :]).then_inc(exta_sem, 16)
            sync.dma_start(ext_b[:, :], exts[128:256, :]).then_inc(extb_sem, 16)
            sync.wait_ge(act1_sem, 1)
            sync.dma_start(bin_a[:, :], r16_a[:, :]).then_inc(b1sem_a, 16)

            for t in range(t_steps):
                par = t % 2
                # gathered r_t (AG output) -> G tile for this step's matmuls
                sync.wait_ge(ccsem, t + 1)
                sync.dma_start(
                    bass.AP(g_t[par], 0, [[512, 128], [64, 8], [1, 64]]),
                    bass.AP(bout_t[par], 0, [[64, 128], [8192, 8], [1, 64]]),
                ).then_inc(sem_g[par], 16)
                # r_{t+1} (sigma of step t) -> AG input bounce for step t+1
                sync.wait_ge(act1_sem, t + 2)
                sync.dma_start(bin_t[1 - par][:, :], r16_t[1 - par][:, :]
                               ).then_inc(b1sem[1 - par], 16)
                # usave[t] = u_{t+1}, produced by DVE into u_t[(t+1)%2]
                sync.wait_ge(dveu_sem, t + 1)
                sync.dma_start(
                    usave[t * 128:(t + 1) * 128, :],
                    u_t[(t + 1) % 2][:, :],
                ).then_inc(usave_sem, 16)
                # prefetch ext for step t+2 into the tile step t consumed
                if t + 2 < t_steps:
                    sync.dma_start(
                        ext_t[par][:, :],
                        exts[(t + 2) * 128:(t + 3) * 128, :],
                    ).then_inc(ext_sem[par], 16)
                # outsave[t]
                sync.wait_ge(dvo_sem, t + 1)
                sync.dma_start(
                    osave[t * 2:(t + 1) * 2, :],
                    osv_t[par][:, :],
                ).then_inc(osv_dma_sem, 16)

            sync.wait_ge(usave_sem, 16 * t_steps)
            sync.wait_ge(osv_dma_sem, 16 * t_steps)

        # ---------------- tensor engine: all matmuls ----------------
        @block.tensor
        def _(tensor):
            def emit_step(par, th_act1, th_g, th_dvo, th_psfree):
                # partial readout for step t from local r16
                tensor.wait_ge(act1_sem, th_act1)
                tensor.wait_ge(dvo_sem, th_dvo)
                tensor.matmul(po_t[par][:, :], wo_sb[:, 0:2],
                              r16_t[par][:, 0:32], start=True, stop=False)
                tensor.matmul(po_t[par][:, :], wo_sb[:, 2:4],
                              r16_t[par][:, 32:64], start=False, stop=True,
                              ).then_inc(rdout_sem, 1)
                # recurrent accumulation: psum = r_t @ W
                tensor.wait_ge(dveu_sem, th_psfree)
                tensor.wait_ge(sem_g[par], th_g)
                first = True
                for jj in range(2):
                    for k in range(NCORES):
                        for j in range(2):
                            last = (jj == 1 and k == NCORES - 1 and j == 1)
                            mm = tensor.matmul(
                                ch(ps, jj),
                                wblk(k, j, jj),
                                g_t[par][:, 64 * k + 32 * j:64 * k + 32 * j + 32],
                                start=first, stop=last)
                            first = False
                            if last:
                                mm.then_inc(psum_sem, 1)

            tensor.wait_ge(init_sem, 64)
            with (
                tensor.register("r_act1") as r_act1,
                tensor.register("r_ga") as r_ga,
                tensor.register("r_gb") as r_gb,
                tensor.register("r_dvo") as r_dvo,
                tensor.register("r_psf") as r_psf,
            ):
                tensor.reg_mov(r_act1, 1)
                tensor.reg_mov(r_ga, 16)
                tensor.reg_mov(r_gb, 16)
                tensor.reg_mov(r_dvo, 0)
                tensor.reg_mov(r_psf, 0)
                with tensor.Fori(0, nbody):
                    emit_step(0, r_act1, r_ga, r_dvo, r_psf)
                    tensor.reg_add(r_act1, r_act1, 1)
                    tensor.reg_add(r_psf, r_psf, 1)
                    emit_step(1, r_act1, r_gb, r_dvo, r_psf)
                    tensor.reg_add(r_act1, r_act1, 1)
                    tensor.reg_add(r_psf, r_psf, 1)
                    tensor.reg_add(r_ga, r_ga, 16)
                    tensor.reg_add(r_gb, r_gb, 16)
                    tensor.reg_add(r_dvo, r_dvo, 2)

        # -------- vector engine (DVE): leaky integration + readout copy ----
        @block.vector
        def _(vector):
            def dve_step(par, regs):
                nxt = 1 - par
                # h = u_t*(1-dt) + dt*(mue+ext_t)   (off critical path)
                vector.wait_ge(ext_sem[par], regs["ext"][par])
                if uniform:
                    vector.scalar_tensor_tensor(
                        h_sb[:, :], u_t[par][:, :], 1.0 - dt_scalar,
                        ext_t[par][:, :], ALU.mult, ALU.add)
                else:
                    for jj in range(2):
                        vector.scalar_tensor_tensor(
                            ch(h_sb, jj), ch(u_t[par], jj),
                            scl_sb[:, 2 + jj:3 + jj], ch(ext_t[par], jj),
                            ALU.mult, ALU.add)
                vector.drain()
                # u_{t+1} = dt*psum + h   (critical path)
                vector.wait_ge(psum_sem, regs["psum"])
                vector.wait_ge(usave_sem, regs["usave"])
                if uniform:
                    vector.scalar_tensor_tensor(
                        u_t[nxt][:, :], ps[:, :], dt_scalar, h_sb[:, :],
                        ALU.mult, ALU.add).then_inc(dveu_sem, 1)
                else:
                    vector.scalar_tensor_tensor(
                        ch(u_t[nxt], 0), ch(ps, 0), scl_sb[:, 0:1],
                        ch(h_sb, 0), ALU.mult, ALU.add)
                    vector.scalar_tensor_tensor(
                        ch(u_t[nxt], 1), ch(ps, 1), scl_sb[:, 1:2],
                        ch(h_sb, 1), ALU.mult, ALU.add).then_inc(dveu_sem, 1)
                vector.drain()
                # readout copy (off critical path)
                vector.wait_ge(rdout_sem, regs["rd"])
                vector.wait_ge(osv_dma_sem, regs["osv"])
                vector.tensor_copy(osv_t[par][:, :], po_t[par][:, :]
                                   ).then_inc(dvo_sem, 1)

            vector.wait_ge(init_sem, 64)
            with (
                vector.register("r_exa") as r_exa,
                vector.register("r_exb") as r_exb,
                vector.register("r_ps") as r_ps,
                vector.register("r_us") as r_us,
                vector.register("r_rd") as r_rd,
                vector.register("r_ov") as r_ov,
            ):
                vector.reg_mov(r_exa, 16)
                vector.reg_mov(r_exb, 16)
                vector.reg_mov(r_ps, 1)
                vector.reg_mov(r_us, 0)
                vector.reg_mov(r_rd, 1)
                vector.reg_mov(r_ov, 0)
                regs = {"ext": [r_exa, r_exb], "psum": r_ps, "usave": r_us,
                        "rd": r_rd, "osv": r_ov}
                with vector.Fori(0, nbody):
                    dve_step(0, regs)
                    vector.reg_add(r_ps, r_ps, 1)
                    vector.reg_add(r_us, r_us, 16)
                    vector.reg_add(r_rd, r_rd, 1)
                    vector.reg_add(r_ov, r_ov, 16)
                    dve_step(1, regs)
                    vector.reg_add(r_ps, r_ps, 1)
                    vector.reg_add(r_us, r_us, 16)
                    vector.reg_add(r_rd, r_rd, 1)
                    vector.reg_add(r_ov, r_ov, 16)
                    vector.reg_add(r_exa, r_exa, 16)
                    vector.reg_add(r_exb, r_exb, 16)

        # ---------------- scalar engine (ACT): sigmoid ----------------
        @block.scalar
        def _(scalar):
            scalar.wait_ge(init_sem, 64)
            # r_0 = sigmoid(3 u0 - 2), fp16
            scalar.activation(r16_a[:, :], u_a[:, :], AF.Sigmoid,
                              bias=scl_sb[:, 4:5], scale=3.0,
                              ).then_inc(act1_sem, 1)
            with (
                scalar.register("r_dvu") as r_dvu,
                scalar.register("r_ba") as r_ba,
                scalar.register("r_bb") as r_bb,
            ):
                scalar.reg_mov(r_dvu, 1)
                scalar.reg_mov(r_ba, 16)
                scalar.reg_mov(r_bb, 0)
                with scalar.Fori(0, nbody):
                    for par in range(2):
                        nxt = 1 - par
                        scalar.wait_ge(dveu_sem, r_dvu)
                        # r16[nxt] was last read by dma1 of step t-1 (same
                        # parity); wait for that DMA before overwriting (WAR)
                        scalar.wait_ge(b1sem[nxt], [r_ba, r_bb][nxt])
                        scalar.activation(r16_t[nxt][:, :], u_t[nxt][:, :],
                                          AF.Sigmoid, bias=scl_sb[:, 4:5],
                                          scale=3.0).then_inc(act1_sem, 1)
                        scalar.reg_add(r_dvu, r_dvu, 1)
                    scalar.reg_add(r_ba, r_ba, 16)
                    scalar.reg_add(r_bb, r_bb, 16)

        # ---------------- gpsimd: per-step AllGather of r16 ----------------
        @block.gpsimd
        def _(gpsimd):
            rg = [list(range(NCORES))]
            for t in range(t_steps):
                par = t % 2
                # AG input bounce for step t written (includes HBM receipt)
                gpsimd.wait_ge(b1sem[par], 16 * (t // 2 + 1))
                if t >= 2:
                    # bout[par] free: dma2 of step t-2 finished reading it
                    gpsimd.wait_ge(sem_g[par], 16 * (t // 2))
                gpsimd.collective_compute(
                    "AllGather",
                    ALU.bypass,
                    replica_groups=rg,
                    ins=[bin_t[par].ap().opt()],
                    outs=[bout_t[par].ap().opt()],
                ).then_inc(ccsem, 1)

    if not nc.is_finalized():
        nc.finalize()
    return nc


# ======================= host-side packing =======================

def _to_tile64(x):
    """[..., B=32, 256] per-core slice -> [..., 128, 64] neuron-major tile."""
    return np.ascontiguousarray(
        x.reshape(x.shape[:-2] + (32, 2, 128)).swapaxes(-1, -3).reshape(
            x.shape[:-2] + (128, 64)))


def _from_tile64(x):
    """[..., 128, 64] tile -> [..., 32, 256]."""
    return np.ascontiguousarray(
        x.reshape(x.shape[:-2] + (128, 2, 32)).swapaxes(-1, -3).reshape(
            x.shape[:-2] + (32, 256)))


def make_in_maps(u0, ext, weight_rec, mask, weight_sign, weight_out,
                 dt_tausinv, t_steps=T):
    u0 = np.asarray(u0, np.float32)
    ext = np.asarray(ext, np.float32)[:t_steps]
    w_dale_t = (np.asarray(mask, np.float32)
                * np.maximum(np.asarray(weight_rec, np.float32), 0.0)
                * np.asarray(weight_sign, np.float32)[None, :]).T  # [pre, post]
    wout_f = np.asarray(weight_out, np.float32)
    dt = np.asarray(dt_tausinv, np.float32).reshape(-1)            # [N]

    in_maps = []
    for c in range(NCORES):
        # recurrent weights, permuted to match the XOR-relative gather layout
        wrec_c = np.empty((128, 32 * 128), np.float32)
        for k in range(NCORES):
            src = k * S
            for j in range(2):
                for jj in range(2):
                    base = ((k * 2 + j) * 2 + jj) * 128
                    wrec_c[:, base:base + 128] = w_dale_t[
                        src + 128 * j: src + 128 * (j + 1),
                        c * S + 128 * jj: c * S + 128 * (jj + 1)]
        # readout weights: cols [2j, 2j+2) = chunk j
        wout_c = np.empty((128, 4), np.float32)
        for j in range(2):
            wout_c[:, 2 * j:2 * j + 2] = wout_f[c * S + 128 * j: c * S + 128 * (j + 1)]
        # per-partition dt scalars (used only when dt is non-uniform) + consts
        scl_c = np.empty((128, 6), np.float32)
        for jj in range(2):
            dloc = dt[c * S + 128 * jj: c * S + 128 * (jj + 1)]
            scl_c[:, jj] = dloc
            scl_c[:, 2 + jj] = 1.0 - dloc
        scl_c[:, 4] = -2.0
        scl_c[:, 5] = 0.0

        dt_t = _to_tile64(dt[None, c * S:(c + 1) * S].repeat(32, axis=0))
        u0t_c = _to_tile64(u0[:, c * S:(c + 1) * S])
        # ext tile pre-scaled: dt * (MUE + ext_t)
        exts_c = dt_t[None] * _to_tile64(MUE + ext[:, :, c * S:(c + 1) * S])

        in_maps.append(dict(
            wrec=wrec_c.astype(np.float16),
            wout=wout_c.astype(np.float16),
            scl=scl_c,
            u0t=u0t_c,
            exts=np.ascontiguousarray(exts_c, np.float32).reshape(
                t_steps * 128, 64),
        ))
    return in_maps


def unpack_outputs(results, t_steps=T):
    usave = np.empty((t_steps, B, N), np.float32)
    osum = np.zeros((t_steps, 2, 32), np.float32)
    for c in range(NCORES):
        uc = np.asarray(results[c]["usave"], np.float32).reshape(t_steps, 128, 64)
        usave[:, :, c * S:(c + 1) * S] = _from_tile64(uc)
        osum += np.asarray(results[c]["osave"], np.float32).reshape(t_steps, 2, 32)
    outsave = np.ascontiguousarray(osum.transpose(0, 2, 1))
    return usave, outsave


_CACHED = {}


def kernel(u0, ext, weight_rec, mask, weight_sign, weight_out, dt_tausinv):
    from concourse.bass_utils import run_bass_kernel_spmd

    dt = np.asarray(dt_tausinv, np.float32).reshape(-1)
    dt_scalar = float(dt[0]) if np.all(dt == dt[0]) else None
    in_maps = make_in_maps(u0, ext, weight_rec, mask, weight_sign,
                           weight_out, dt_tausinv)
    key = (T, dt_scalar)
    if key not in _CACHED:
        _CACHED[key] = build_nc(t_steps=T, dt_scalar=dt_scalar)
    res = run_bass_kernel_spmd(_CACHED[key], in_maps,
                               core_ids=list(range(NCORES)))
    return unpack_outputs(res.results)
